# revision 1
# baseline (speedup 1.0000x reference)
"""TextCNN-style conv layer (kernel sizes 3/4/5, EMB=300 -> DEPTH=256, bias,
ReLU, max-pool over time) as a Bass/Tile kernel for 8 Trainium2 NeuronCores.

Strategy: data-parallel over batch (8 samples per core), weights replicated.

Conv as dense-K matmuls: for branch n, window output
y[d, i] = sum_{k < n*300} Xrep[k, i] * Wn[d, k]  with  Xrep[k, i] =
x[i + k//300, k%300] -- the im2col matrix.  Xrep rows are materialized in
SBUF as 12 K-tiles of 128 rows per sample, built by <=2 shifted DMA segments
per tile straight from the transposed input in DRAM (a row (j, e) is just
x_t[e, j:] -- a free-dim offset), so no host-side replication and each
branch contracts over ceil(n*300/128) dense K=128 tiles (8/10/12 -> 30
matmuls per sample per depth-half vs 36 for the per-(j,chunk) split).
Branch boundaries that fall inside a tile are handled by zero-padding the
*weights* (the x rows there hold valid shifted data).  The final K-tile's
rows past 1500 are never written, so its matmuls contract only K=92.

dtype float32r: FP22 multiplies at full PE rate, fp32 PSUM accumulate; the
moving free-dim count must be even, so branches with odd SEQ-n compute one
extra (still valid) window that the max-reduce then ignores.

Epilogue: relu(max_i(y + b)) == max(0, max_i y + b): DVE reduce_max over the
window axis straight out of PSUM, broadcast bias add + clamp at 0, output
staged [d, branch, half, sample] per core and de-transposed on host.
"""

import numpy as np

B, SEQ, EMB = 64, 394, 300
DEPTH = 256
NCORES = 8
BPC = B // NCORES  # samples per core
SEQP = 400  # x_t free-dim padded (zeros) so shifted loads stay in bounds
NS = (3, 4, 5)
NTILES = (8, 10, 12)  # ceil(n*300/128) K-tiles per branch
COLB = (0, 8, 18)  # weight column base per branch
NCOL = 30
KTOT = 12  # distinct Xrep K-tiles per sample

# DMA segments building the 12 Xrep K-tiles: (tile r, p0, plen, j, e0)
_SEGS = []
for _r in range(KTOT):
    _k, _k1 = 128 * _r, min(128 * (_r + 1), 5 * EMB)
    while _k < _k1:
        _j, _e = divmod(_k, EMB)
        _plen = min(_k1 - _k, EMB - _e)
        _SEGS.append((_r, _k - 128 * _r, _plen, _j, _e))
        _k += _plen

TRACE = False
LAST_RESULT = None

_built = None


def _build_bass():
    import concourse.mybir as mybir
    import concourse.tile as tile
    from concourse import bacc
    from contextlib import ExitStack

    f32 = mybir.dt.float32
    f32r = mybir.dt.float32r
    f16 = mybir.dt.float16

    nc = bacc.Bacc("TRN2", target_bir_lowering=False)
    xt_d = nc.dram_tensor("xt", (BPC, KTOT, 128, SEQP), f16, kind="ExternalInput")
    wq_d = nc.dram_tensor("wq", (128, 2, NCOL, 128), f16, kind="ExternalInput")
    bp_d = nc.dram_tensor("bp", (128, 3, 2), f32, kind="ExternalInput")
    out_d = nc.dram_tensor("out_t", (128, 3, 2, BPC), f32, kind="ExternalOutput")

    with tile.TileContext(nc) as tc, ExitStack() as ctx:
        xpool = ctx.enter_context(tc.tile_pool(name="x", bufs=5))
        wpool = ctx.enter_context(tc.tile_pool(name="w", bufs=1))
        cpool = ctx.enter_context(tc.tile_pool(name="consts", bufs=1))
        spool = ctx.enter_context(tc.tile_pool(name="stage", bufs=1))
        pspool = ctx.enter_context(tc.tile_pool(name="ps", bufs=8, space="PSUM"))

        # x segments alternate over the two fast HWDGE rings (SP, ACT);
        # weights + bias go on the gpsimd SWDGE ring in parallel so they
        # never block the x pipeline.
        hw_engines = (nc.sync, nc.scalar)
        rr = [0]

        wts = {}

        def load_w(dh, br, eng):
            nt = NTILES[br]
            wt = wpool.tile([128, nt, 128], f16, tag=f"w{dh}{br}")
            eng.dma_start(wt[:], wq_d[:, dh, COLB[br] : COLB[br] + nt, :])
            wts[dh, br] = wt

        def load_x(s):
            # One pool tile + one contiguous DMA per K-tile: a matmul waits
            # only on the single DMA that wrote its contraction rows.
            xr = [
                xpool.tile([128, SEQP], f16, tag=f"x{r}", name=f"x{r}_{s}")
                for r in range(KTOT)
            ]
            for r in range(KTOT):
                eng = hw_engines[rr[0] % 2]
                rr[0] += 1
                eng.dma_start(xr[r][:], xt_d[s, r])
            return xr

        # The whole working set (3.9MB weights + 2.4MB im2col per sample)
        # drains from HBM at ~350GB/s, so the first ~25us are DMA-paced.
        # Interleave the loads in need-order and run samples 0-1 group-major
        # (stretching each weight tile's deadline) before switching to
        # sample-major for the pipelined steady state.
        load_w(0, 0, nc.sync)
        xrs = [load_x(0)]
        load_w(0, 1, nc.scalar)
        load_w(0, 2, nc.sync)
        load_w(1, 0, nc.scalar)
        load_w(1, 1, nc.sync)
        load_w(1, 2, nc.scalar)
        xrs.append(load_x(1))
        bt = cpool.tile([128, 3, 2], f32)
        nc.gpsimd.dma_start(bt[:], bp_d[:])
        xrs.append(load_x(2))

        stage = spool.tile([128, 3, 2, BPC], f32)

        def do_group(s, dh, br):
            n = NS[br]
            nw = SEQ - n  # windows the reference maxes over
            nmm = nw + (nw & 1)  # keep the moving count even
            nt = NTILES[br]
            xr = xrs[s]
            ps = pspool.tile([128, 512], f32, tag="ps", name=f"ps_{s}_{dh}_{br}")
            for r in range(nt):
                kk = min(128, 5 * EMB - 128 * r)  # 92 on the last tile
                nc.tensor.matmul(
                    ps[:, :nmm],
                    lhsT=wts[dh, br][:kk, r, :],
                    rhs=xr[r][:kk, :nmm],
                    start=(r == 0),
                    stop=(r == nt - 1),
                )
            nc.vector.reduce_max(
                stage[:, br, dh, s : s + 1],
                ps[:, :nw],
                axis=mybir.AxisListType.X,
            )

        for s in range(BPC):
            if 3 <= s + 3 < BPC + 3 and s + 3 < BPC:
                xrs.append(load_x(s + 3))
            for dh in range(2):
                for br in range(3):
                    do_group(s, dh, br)

        stage2 = spool.tile([128, 3, 2, BPC], f32)
        nc.vector.tensor_tensor(
            stage2[:],
            stage[:],
            bt[:, :, :, None].to_broadcast((128, 3, 2, BPC)),
            mybir.AluOpType.add,
        )
        nc.vector.tensor_scalar_max(stage2[:], stage2[:], 0.0)
        nc.sync.dma_start(out_d[:], stage2[:])

    nc.compile()
    return nc


def _pack_inputs(input, W1, W2, W3, b1, b2, b3):
    # Host-materialized im2col: Xrep[b, k, t] = x[b, t + k//300, k%300],
    # laid out as 12 K-tiles of 128 rows, SEQ padded to 400 with zeros.
    xt = np.zeros((B, EMB, SEQP), np.float32)
    xt[:, :, :SEQ] = np.asarray(input, np.float32).transpose(0, 2, 1)
    xrep = np.zeros((B, KTOT * 128, SEQP), np.float32)
    for j in range(5):
        rows = xrep[:, j * EMB : (j + 1) * EMB, : SEQP - j]
        rows[:] = xt[:, :, j:]
    xt = xrep.reshape(B, KTOT, 128, SEQP).astype(np.float16)

    wq = np.zeros((128, 2, NCOL, 128), np.float32)  # cast to fp16 below
    for br, (n, W) in enumerate(zip(NS, (W1, W2, W3))):
        Wt = np.asarray(W, np.float32).T  # [n*300, 256]
        for r in range(NTILES[br]):
            rows = Wt[128 * r : min(128 * (r + 1), n * EMB)]
            for dh in range(2):
                wq[: rows.shape[0], dh, COLB[br] + r, :] = (
                    rows[:, dh * 128 : (dh + 1) * 128]
                )

    wq = wq.astype(np.float16)

    bp = np.empty((128, 3, 2), np.float32)
    for br, b in enumerate((b1, b2, b3)):
        b = np.asarray(b, np.float32).reshape(DEPTH)
        for dh in range(2):
            bp[:, br, dh] = b[dh * 128 : (dh + 1) * 128]
    return xt, wq, bp


def kernel(input, W1, W2, W3, b1, b2, b3):
    global _built, LAST_RESULT
    from concourse.bass_utils import run_bass_kernel_spmd

    xt, wq, bp = _pack_inputs(input, W1, W2, W3, b1, b2, b3)

    if _built is None:
        _built = _build_bass()
    nc = _built

    in_maps = [
        {"xt": xt[c * BPC : (c + 1) * BPC], "wq": wq, "bp": bp}
        for c in range(NCORES)
    ]
    res = run_bass_kernel_spmd(
        nc, in_maps, core_ids=list(range(NCORES)), trace=TRACE
    )
    LAST_RESULT = res

    out = np.empty((B, 3 * DEPTH), np.float32)
    for c in range(NCORES):
        arr = res.results[c]["out_t"]  # [128, 3, 2, BPC]
        out[c * BPC : (c + 1) * BPC] = arr.transpose(3, 1, 2, 0).reshape(BPC, 768)
    return out



# revision 6
# speedup vs baseline: 1.7020x; 1.7020x over previous
"""TextCNN-style conv layer (kernel sizes 3/4/5, EMB=300 -> DEPTH=256, bias,
ReLU, max-pool over time) as a Bass/Tile kernel for 8 Trainium2 NeuronCores.

Strategy: data-parallel over batch (8 samples per core), weights replicated.

Conv as dense-K matmuls over the im2col matrix Xrep[k, i] = x[i + k//300,
k%300], exactly as the bf16 version -- but in fp8 e4m3 with DoubleRow
matmuls: each MM contracts a PAIR of 128-row K-subtiles (virtual K=256, two
fp8 weights per PE cell, 2 MAC/cell/cycle), halving the matmul count to
4/5/6 pairs per branch = 15 per (sample, depth-half), 240 per core.  Both
operands use 3D APs [128, 2, N] whose middle dim indexes the subtile pair
(tile_matmul's layout).  e4m3 quantization of both operands measures
L2 rel err 1.07e-2 on the seed-0 data (gate 2e-2); max-pool over ~390
windows keeps outputs at ~3 sigma so the relative error stays small.

Schedule: the whole working set is tiny in fp8 (4.9MB x + 1MB w per core),
so everything is prefetched up-front on the two HWDGE rings; a short burst
of throwaway bf16 matmuls on a memset tile keeps the PE busy from t~=0.3us
so the HAM clock-gate warms (K=8/8) while the first sample's DMA lands,
instead of the 12us idle + 24us half-clock start the bf16 version paid.
PSUM: 7 banks round-robin real accumulation groups, 1 bank for the dummies.

Epilogue unchanged: DVE reduce_max over the window axis straight out of
PSUM, broadcast bias add + clamp at 0, output staged [d, branch, half,
sample] per core and de-transposed on host.
"""

import numpy as np
import ml_dtypes

B, SEQ, EMB = 64, 394, 300
DEPTH = 256
NCORES = 8
BPC = B // NCORES  # samples per core
SEQP = 400  # im2col free-dim padded (zeros) so all windows exist
NS = (3, 4, 5)
NPAIRS = (4, 5, 6)  # DoubleRow K-pair count per branch (ceil(n*300/256))
PRB = (0, 4, 9)  # weight pair-slot base per branch
NPR = 15  # total weight pair slots
KTOT = 12  # 128-row K-subtiles of the im2col per sample

NDUMMY = 6  # bf16 warm-up matmuls (N=512, ~2.6us cold) to spin up HAM

TRACE = False
LAST_RESULT = None

_built = None


def _build_bass():
    import concourse.mybir as mybir
    import concourse.tile as tile
    from concourse import bacc
    from contextlib import ExitStack

    f32 = mybir.dt.float32
    f8 = mybir.dt.float8e4
    bf16 = mybir.dt.bfloat16
    DR = mybir.MatmulPerfMode.DoubleRow

    nc = bacc.Bacc("TRN2", target_bir_lowering=False)
    xt_d = nc.dram_tensor("xt", (BPC, 128, KTOT, SEQP), f8, kind="ExternalInput")
    wq_d = nc.dram_tensor("wq", (128, 2, NPR, 2, 128), f8, kind="ExternalInput")
    bp_d = nc.dram_tensor("bp", (128, 3, 2), f32, kind="ExternalInput")
    out_d = nc.dram_tensor("out_t", (128, 3, 2, BPC), f32, kind="ExternalOutput")

    with tile.TileContext(nc) as tc, ExitStack() as ctx:
        xpool = ctx.enter_context(tc.tile_pool(name="x", bufs=1))
        spool = ctx.enter_context(tc.tile_pool(name="stage", bufs=1))
        pspool = ctx.enter_context(tc.tile_pool(name="ps", bufs=1, space="PSUM"))

        # Warm-up fodder: PE busy from the first instruction while DMAs land.
        dmy = spool.tile([128, 512], bf16, tag="dmy")
        nc.vector.memset(dmy[:], 0.0)
        psd = pspool.tile([128, 512], f32, tag="dmy", bufs=1)
        for _ in range(NDUMMY):
            nc.tensor.matmul(psd[:], lhsT=dmy[:, :128], rhs=dmy[:], start=True,
                             stop=True)

        # Prefetch everything: sample 0 + dh0 weights first (gate the first
        # real MM), then the rest round-robin over the two fast HWDGE rings.
        xs = [
            xpool.tile([128, KTOT, SEQP], f8, tag=f"x{s}", name=f"x{s}")
            for s in range(BPC)
        ]
        wt = [
            xpool.tile([128, NPR, 2, 128], f8, tag=f"w{dh}", name=f"w{dh}")
            for dh in range(2)
        ]
        bt = spool.tile([128, 3, 2], f32, tag="bt")
        nc.sync.dma_start(xs[0][:], xt_d[0])
        nc.scalar.dma_start(wt[0][:], wq_d[:, 0])
        nc.sync.dma_start(wt[1][:], wq_d[:, 1])
        nc.scalar.dma_start(xs[1][:], xt_d[1])
        for s in range(2, BPC):
            (nc.sync, nc.scalar)[s % 2].dma_start(xs[s][:], xt_d[s])
        nc.scalar.dma_start(bt[:], bp_d[:])

        stage = spool.tile([128, 3, 2, BPC], f32, tag="stage")

        def do_group(s, dh, br):
            n = NS[br]
            nw = SEQ - n  # windows the reference maxes over
            nmm = nw + (nw & 1)  # keep the moving count even
            np_ = NPAIRS[br]
            ps = pspool.tile([128, 512], f32, tag="ps", bufs=7,
                             name=f"ps_{s}_{dh}_{br}")
            for j in range(np_):
                nc.tensor.matmul(
                    ps[:, :nmm],
                    lhsT=wt[dh][:, PRB[br] + j, :, :],
                    rhs=xs[s][:, 2 * j : 2 * j + 2, :nmm],
                    start=(j == 0),
                    stop=(j == np_ - 1),
                    perf_mode=DR,
                )
            nc.vector.reduce_max(
                stage[:, br, dh, s : s + 1],
                ps[:, :nw],
                axis=mybir.AxisListType.X,
            )

        for s in range(BPC):
            for dh in range(2):
                for br in range(3):
                    do_group(s, dh, br)

        stage2 = spool.tile([128, 3, 2, BPC], f32, tag="stage2")
        nc.vector.tensor_tensor(
            stage2[:],
            stage[:],
            bt[:, :, :, None].to_broadcast((128, 3, 2, BPC)),
            mybir.AluOpType.add,
        )
        nc.vector.tensor_scalar_max(stage2[:], stage2[:], 0.0)
        nc.sync.dma_start(out_d[:], stage2[:])

    nc.compile()
    return nc


def _pack_inputs(input, W1, W2, W3, b1, b2, b3):
    # Host-materialized im2col: Xrep[b, k, t] = x[b, t + k//300, k%300],
    # 12 K-subtiles of 128 rows, SEQ padded to 400 with zeros, laid out
    # [sample, partition, subtile, t] so each sample loads in one DMA.
    f8 = ml_dtypes.float8_e4m3
    xt = np.zeros((B, EMB, SEQP), np.float32)
    xt[:, :, :SEQ] = np.asarray(input, np.float32).transpose(0, 2, 1)
    xrep = np.zeros((B, KTOT * 128, SEQP), np.float32)
    for j in range(5):
        xrep[:, j * EMB : (j + 1) * EMB, : SEQP - j] = xt[:, :, j:]
    xt8 = np.ascontiguousarray(
        xrep.reshape(B, KTOT, 128, SEQP).transpose(0, 2, 1, 3)
    ).astype(f8)

    # Weights: [partition, depth-half, pair-slot, pair-member, depth-col],
    # branch sections at PRB, rows zero-padded past each branch's n*300.
    wq = np.zeros((128, 2, NPR, 2, 128), np.float32)
    for br, (n, W) in enumerate(zip(NS, (W1, W2, W3))):
        Wt = np.asarray(W, np.float32).T  # [n*300, 256]
        for u in range(2 * NPAIRS[br]):
            rows = Wt[128 * u : min(128 * (u + 1), n * EMB)]
            if rows.shape[0] == 0:
                continue
            for dh in range(2):
                wq[: rows.shape[0], dh, PRB[br] + u // 2, u % 2, :] = (
                    rows[:, dh * 128 : (dh + 1) * 128]
                )
    wq8 = wq.astype(f8)

    bp = np.empty((128, 3, 2), np.float32)
    for br, b in enumerate((b1, b2, b3)):
        b = np.asarray(b, np.float32).reshape(DEPTH)
        for dh in range(2):
            bp[:, br, dh] = b[dh * 128 : (dh + 1) * 128]
    return xt8, wq8, bp


def kernel(input, W1, W2, W3, b1, b2, b3):
    global _built, LAST_RESULT
    from concourse.bass_utils import run_bass_kernel_spmd

    xt8, wq8, bp = _pack_inputs(input, W1, W2, W3, b1, b2, b3)

    if _built is None:
        _built = _build_bass()
    nc = _built

    in_maps = [
        {"xt": xt8[c * BPC : (c + 1) * BPC], "wq": wq8, "bp": bp}
        for c in range(NCORES)
    ]
    res = run_bass_kernel_spmd(
        nc, in_maps, core_ids=list(range(NCORES)), trace=TRACE
    )
    LAST_RESULT = res

    out = np.empty((B, 3 * DEPTH), np.float32)
    for c in range(NCORES):
        arr = res.results[c]["out_t"]  # [128, 3, 2, BPC]
        out[c * BPC : (c + 1) * BPC] = arr.transpose(3, 1, 2, 0).reshape(BPC, 768)
    return out


# revision 7
# speedup vs baseline: 1.7780x; 1.0447x over previous
"""TextCNN-style conv layer (kernel sizes 3/4/5, EMB=300 -> DEPTH=256, bias,
ReLU, max-pool over time) as a Bass/Tile kernel for 8 Trainium2 NeuronCores.

Strategy: data-parallel over batch (8 samples per core), weights replicated.

Conv as dense-K matmuls over the im2col matrix Xrep[k, i] = x[i + k//300,
k%300] in fp8 e4m3 with DoubleRow matmuls: each MM contracts a PAIR of
128-row K-subtiles (virtual K=256, two fp8 weights per PE cell), so each
branch needs 4/5/6 pair-MMs = 15 per (sample, depth-half), 240 per core --
half the bf16 count, and HW paces them at the same ~165ns (N/2.4GHz), so
the PE stream floor drops from 79us to 39.5us.  e4m3 on both operands
measures L2 rel err 1.25e-2 on the seed-0 data (gate 2e-2).

Schedule (from trace analysis of the bf16 + v1 fp8 runs):
- The whole fp8 working set (4.9MB x + 1MB w per core) is prefetched in
  deadline order, split so the first-needed slices land first: sample-0
  x in 3 subtile-range DMAs on the sync ring, weights in 6 branch-slot
  DMAs on the scalar ring (consumers gate on the exact DMA that wrote
  their slice via subtile deps).  Remaining samples alternate rings.
- 7 throwaway bf16 matmuls on a memset tile keep the PE busy from ~7.5us
  (kernel-start) until the first real MM's data lands (~10us), so the HAM
  clock-gate reaches K=8/8 before/just after real MMs begin and the 12us
  idle + 24us half-clock start of the bf16 version is gone.
- PSUM: 7 banks round-robin the 48 accumulation groups, 1 for dummies.
- Epilogue per sample: after a sample's 6 reduce_max groups, a [128,3,2]
  bias-add + relu and its own contiguous output DMA, so the tail after
  the last MM is one group's reduce + 6-element DVE ops + one small DMA.
  (Teardown ~7.5us + preamble ~1.5us are fixed framework costs: the
  semaphore-reset sweep is 57-59 sems/engine for ANY kernel here.)
"""

import numpy as np
import ml_dtypes

B, SEQ, EMB = 64, 394, 300
DEPTH = 256
NCORES = 8
BPC = B // NCORES  # samples per core
SEQP = 400  # im2col free-dim padded (zeros) so all windows exist
NS = (3, 4, 5)
NPAIRS = (4, 5, 6)  # DoubleRow K-pair count per branch (ceil(n*300/256))
PRB = (0, 4, 9)  # weight pair-slot base per branch
NPR = 15  # total weight pair slots
KTOT = 12  # 128-row K-subtiles of the im2col per sample

NDUMMY = 7  # bf16 warm-up matmuls (N=512, ~3us cold) to spin up HAM

TRACE = False
LAST_RESULT = None

_built = None


def _build_bass():
    import concourse.mybir as mybir
    import concourse.tile as tile
    from concourse import bacc
    from contextlib import ExitStack

    f32 = mybir.dt.float32
    f8 = mybir.dt.float8e4
    bf16 = mybir.dt.bfloat16
    DR = mybir.MatmulPerfMode.DoubleRow

    nc = bacc.Bacc("TRN2", target_bir_lowering=False)
    xt_d = nc.dram_tensor("xt", (BPC, 128, KTOT, SEQP), f8, kind="ExternalInput")
    wq_d = nc.dram_tensor("wq", (128, 2, NPR, 2, 128), f8, kind="ExternalInput")
    bp_d = nc.dram_tensor("bp", (128, 3, 2), f32, kind="ExternalInput")
    out_d = nc.dram_tensor("out_t", (BPC, 128, 3, 2), f32, kind="ExternalOutput")

    with tile.TileContext(nc) as tc, ExitStack() as ctx:
        xpool = ctx.enter_context(tc.tile_pool(name="x", bufs=1))
        spool = ctx.enter_context(tc.tile_pool(name="stage", bufs=1))
        pspool = ctx.enter_context(tc.tile_pool(name="ps", bufs=1, space="PSUM"))

        # Warm-up fodder: PE busy from the first possible slot while DMAs
        # land (gpsimd memset so the DVE isn't on the critical path).
        dmy = spool.tile([128, 512], bf16, tag="dmy")
        nc.gpsimd.memset(dmy[:], 0.0)
        psd = pspool.tile([128, 512], f32, tag="dmy", bufs=1)
        for _ in range(NDUMMY):
            nc.tensor.matmul(psd[:], lhsT=dmy[:, :128], rhs=dmy[:], start=True,
                             stop=True)

        xs = [
            xpool.tile([128, KTOT, SEQP], f8, tag=f"x{s}", name=f"x{s}")
            for s in range(BPC)
        ]
        wt = [
            xpool.tile([128, NPR, 2, 128], f8, tag=f"w{dh}", name=f"w{dh}")
            for dh in range(2)
        ]
        bt = spool.tile([128, 3, 2], f32, tag="bt")
        nc.gpsimd.dma_start(bt[:], bp_d[:])

        # Deadline-ordered prefetch.  Sync ring: sample-0 x in 3 chunks,
        # then odd samples.  Scalar ring: the 6 weight chunks in the order
        # the (dh, br) groups consume them, then even samples.
        nc.sync.dma_start(xs[0][:, 0:4], xt_d[0, :, 0:4])
        nc.scalar.dma_start(wt[0][:, 0:4], wq_d[:, 0, 0:4])
        nc.sync.dma_start(xs[0][:, 4:8], xt_d[0, :, 4:8])
        nc.scalar.dma_start(wt[0][:, 4:9], wq_d[:, 0, 4:9])
        nc.sync.dma_start(xs[0][:, 8:12], xt_d[0, :, 8:12])
        nc.scalar.dma_start(wt[0][:, 9:15], wq_d[:, 0, 9:15])
        nc.scalar.dma_start(wt[1][:, 0:4], wq_d[:, 1, 0:4])
        nc.scalar.dma_start(wt[1][:, 4:9], wq_d[:, 1, 4:9])
        nc.scalar.dma_start(wt[1][:, 9:15], wq_d[:, 1, 9:15])
        nc.sync.dma_start(xs[1][:], xt_d[1])
        nc.scalar.dma_start(xs[2][:], xt_d[2])
        nc.sync.dma_start(xs[3][:], xt_d[3])
        nc.scalar.dma_start(xs[4][:], xt_d[4])
        nc.sync.dma_start(xs[5][:], xt_d[5])
        nc.scalar.dma_start(xs[6][:], xt_d[6])
        nc.sync.dma_start(xs[7][:], xt_d[7])

        stage = spool.tile([128, BPC, 3, 2], f32, tag="stage")
        stage2 = spool.tile([128, BPC, 3, 2], f32, tag="stage2")

        def do_group(s, dh, br):
            n = NS[br]
            nw = SEQ - n  # windows the reference maxes over
            nmm = nw + (nw & 1)  # keep the moving count even
            np_ = NPAIRS[br]
            ps = pspool.tile([128, 512], f32, tag="ps", bufs=7,
                             name=f"ps_{s}_{dh}_{br}")
            for j in range(np_):
                nc.tensor.matmul(
                    ps[:, :nmm],
                    lhsT=wt[dh][:, PRB[br] + j, :, :],
                    rhs=xs[s][:, 2 * j : 2 * j + 2, :nmm],
                    start=(j == 0),
                    stop=(j == np_ - 1),
                    perf_mode=DR,
                )
            nc.vector.reduce_max(
                stage[:, s, br, dh : dh + 1],
                ps[:, :nw],
                axis=mybir.AxisListType.X,
            )

        for s in range(BPC):
            for dh in range(2):
                for br in range(3):
                    do_group(s, dh, br)
            # Per-sample epilogue + its own small contiguous output DMA.
            nc.vector.tensor_tensor(
                stage2[:, s], stage[:, s], bt[:], mybir.AluOpType.add
            )
            nc.vector.tensor_scalar_max(stage2[:, s], stage2[:, s], 0.0)
            (nc.sync, nc.scalar)[s % 2].dma_start(out_d[s], stage2[:, s])

    nc.compile()
    return nc


def _pack_inputs(input, W1, W2, W3, b1, b2, b3):
    # Host-materialized im2col: Xrep[b, k, t] = x[b, t + k//300, k%300],
    # 12 K-subtiles of 128 rows, SEQ padded to 400 with zeros, laid out
    # [sample, partition, subtile, t] so a sample loads in few big DMAs.
    f8 = ml_dtypes.float8_e4m3
    xt = np.zeros((B, EMB, SEQP), np.float32)
    xt[:, :, :SEQ] = np.asarray(input, np.float32).transpose(0, 2, 1)
    xrep = np.zeros((B, KTOT * 128, SEQP), np.float32)
    for j in range(5):
        xrep[:, j * EMB : (j + 1) * EMB, : SEQP - j] = xt[:, :, j:]
    xt8 = np.ascontiguousarray(
        xrep.reshape(B, KTOT, 128, SEQP).transpose(0, 2, 1, 3)
    ).astype(f8)

    # Weights: [partition, depth-half, pair-slot, pair-member, depth-col],
    # branch sections at PRB, rows zero-padded past each branch's n*300.
    wq = np.zeros((128, 2, NPR, 2, 128), np.float32)
    for br, (n, W) in enumerate(zip(NS, (W1, W2, W3))):
        Wt = np.asarray(W, np.float32).T  # [n*300, 256]
        for u in range(2 * NPAIRS[br]):
            rows = Wt[128 * u : min(128 * (u + 1), n * EMB)]
            if rows.shape[0] == 0:
                continue
            for dh in range(2):
                wq[: rows.shape[0], dh, PRB[br] + u // 2, u % 2, :] = (
                    rows[:, dh * 128 : (dh + 1) * 128]
                )
    wq8 = wq.astype(f8)

    bp = np.empty((128, 3, 2), np.float32)
    for br, b in enumerate((b1, b2, b3)):
        b = np.asarray(b, np.float32).reshape(DEPTH)
        for dh in range(2):
            bp[:, br, dh] = b[dh * 128 : (dh + 1) * 128]
    return xt8, wq8, bp


def kernel(input, W1, W2, W3, b1, b2, b3):
    global _built, LAST_RESULT
    from concourse.bass_utils import run_bass_kernel_spmd

    xt8, wq8, bp = _pack_inputs(input, W1, W2, W3, b1, b2, b3)

    if _built is None:
        _built = _build_bass()
    nc = _built

    in_maps = [
        {"xt": xt8[c * BPC : (c + 1) * BPC], "wq": wq8, "bp": bp}
        for c in range(NCORES)
    ]
    res = run_bass_kernel_spmd(
        nc, in_maps, core_ids=list(range(NCORES)), trace=TRACE
    )
    LAST_RESULT = res

    out = np.empty((B, 3 * DEPTH), np.float32)
    for c in range(NCORES):
        arr = res.results[c]["out_t"]  # [BPC, 128, 3, 2]
        out[c * BPC : (c + 1) * BPC] = arr.transpose(0, 2, 3, 1).reshape(BPC, 768)
    return out
